# revision 7
# baseline (speedup 1.0000x reference)
import numpy as np
import jax
import jax.numpy as jnp
from functools import partial

# Problem constants (nn_MemoryAsLayerTitan): hardcoded per contract.
B, L, D = 2, 2048, 512
P, M = 32, 1024
H, HD = 8, 64
WIN = 256
EPS = 1e-5
S = P + L  # 2080

N_DEV = 8  # shard: batch (2) x sequence (4)


def _layer_norm(x, g, b):
    mu = jnp.mean(x, axis=-1, keepdims=True)
    var = jnp.mean(jnp.square(x - mu), axis=-1, keepdims=True)
    return (x - mu) * jax.lax.rsqrt(var + EPS) * g + b


def _forward(x, persistent_memory, mem_keys, mem_Wk, mem_Wv, mem_Wq,
             attn_Wq, attn_Wk, attn_Wv, attn_Wo,
             ln1_g, ln1_b, ln2_g, ln2_b, out_W, out_b):
    b = x.shape[0]
    pm = jnp.broadcast_to(persistent_memory[None], (b, P, D))
    combined = jnp.concatenate([pm, x], axis=1)

    # NeuralMemory write then read. exp without max-subtraction is safe here
    # (scores ~ N(0,1)) and keeps the einsum chain GSPMD-friendly.
    k = combined @ mem_Wk
    v = combined @ mem_Wv
    w_write = jax.nn.softmax(jnp.einsum('bsd,md->bsm', k, mem_keys), axis=-1)
    state = jnp.einsum('bsm,bsd->bmd', w_write, v)
    q = combined @ mem_Wq
    w_read = jax.nn.softmax(jnp.einsum('bsd,md->bsm', q, mem_keys), axis=-1)
    mem_out = jnp.einsum('bsm,bmd->bsd', w_read, state)

    h = _layer_norm(mem_out, ln1_g, ln1_b)

    qh = (h @ attn_Wq).reshape(b, S, H, HD)
    kh = (h @ attn_Wk).reshape(b, S, H, HD)
    vh = (h @ attn_Wv).reshape(b, S, H, HD)
    bf = jnp.bfloat16
    scores = jnp.einsum('bqhd,bkhd->bhqk', qh.astype(bf), kh.astype(bf),
                        preferred_element_type=jnp.float32) / np.sqrt(HD)
    idx = jnp.arange(S)
    band = jnp.abs(idx[:, None] - idx[None, :]) < WIN
    scores = jnp.where(band[None, None], scores, jnp.finfo(scores.dtype).min)
    attn = jax.nn.softmax(scores, axis=-1)
    ao = jnp.einsum('bhqk,bkhd->bqhd', attn.astype(bf), vh.astype(bf),
                    preferred_element_type=jnp.float32).reshape(b, S, H * HD)
    ao = ao @ attn_Wo

    ao = _layer_norm(ao, ln2_g, ln2_b)
    return ao @ out_W + out_b


_JIT_CACHE = {}


def _get_fn():
    if "fn" in _JIT_CACHE:
        return _JIT_CACHE["fn"]

    devs = jax.devices()
    use_shard = len(devs) >= N_DEV
    if use_shard:
        try:
            from jax.sharding import Mesh, NamedSharding, PartitionSpec as Ps
            mesh = Mesh(np.array(devs[:N_DEV]).reshape(2, 4), ("b", "s"))
            repl = NamedSharding(mesh, Ps())
            x_sh = NamedSharding(mesh, Ps("b", "s", None))
            in_shardings = (x_sh,) + (repl,) * 15
            out_sharding = NamedSharding(mesh, Ps("b", "s", None))
            fn = jax.jit(_forward, in_shardings=in_shardings,
                         out_shardings=out_sharding)
            _JIT_CACHE["fn"] = fn
            _JIT_CACHE["sharded"] = True
            _JIT_CACHE["in_shardings"] = in_shardings
            return fn
        except Exception:
            pass
    fn = jax.jit(_forward)
    _JIT_CACHE["fn"] = fn
    _JIT_CACHE["sharded"] = False
    return fn


def kernel(**inputs) -> np.ndarray:
    fn = _get_fn()
    order = ["x", "persistent_memory", "mem_keys", "mem_Wk", "mem_Wv",
             "mem_Wq", "attn_Wq", "attn_Wk", "attn_Wv", "attn_Wo",
             "ln1_g", "ln1_b", "ln2_g", "ln2_b", "out_W", "out_b"]
    args = [jnp.asarray(inputs[name]) for name in order]
    out = None
    for attempt in range(2):  # axon can transiently fail ("mesh desynced")
        try:
            out = np.asarray(jax.block_until_ready(fn(*args)))
            break
        except Exception:
            continue
    if out is None:
        # Fallback: single-device jit (always correct).
        fn = jax.jit(_forward)
        _JIT_CACHE["fn"] = fn
        out = np.asarray(jax.block_until_ready(fn(*args)))
    return out.astype(np.float32)


# revision 8
# speedup vs baseline: 2.0020x; 2.0020x over previous
import numpy as np
import jax
import jax.numpy as jnp
from functools import partial

# Problem constants (nn_MemoryAsLayerTitan): hardcoded per contract.
B, L, D = 2, 2048, 512
P, M = 32, 1024
H, HD = 8, 64
WIN = 256
EPS = 1e-5
S = P + L  # 2080

N_DEV = 8  # shard: batch (2) x sequence (4)


def _layer_norm(x, g, b):
    mu = jnp.mean(x, axis=-1, keepdims=True)
    var = jnp.mean(jnp.square(x - mu), axis=-1, keepdims=True)
    return (x - mu) * jax.lax.rsqrt(var + EPS) * g + b


def _forward(x, persistent_memory, mem_keys, mem_Wk, mem_Wv, mem_Wq,
             attn_Wq, attn_Wk, attn_Wv, attn_Wo,
             ln1_g, ln1_b, ln2_g, ln2_b, out_W, out_b):
    b = x.shape[0]
    pm = jnp.broadcast_to(persistent_memory[None], (b, P, D))
    combined = jnp.concatenate([pm, x], axis=1)

    # NeuralMemory write then read. exp without max-subtraction is safe here
    # (scores ~ N(0,1)) and keeps the einsum chain GSPMD-friendly.
    bf = jnp.bfloat16
    f32 = jnp.float32
    k = combined @ mem_Wk
    v = combined @ mem_Wv
    w_write = jax.nn.softmax(
        jnp.einsum('bsd,md->bsm', k.astype(bf), mem_keys.astype(bf),
                   preferred_element_type=f32), axis=-1)
    state = jnp.einsum('bsm,bsd->bmd', w_write.astype(bf), v.astype(bf),
                       preferred_element_type=f32)
    q = combined @ mem_Wq
    w_read = jax.nn.softmax(
        jnp.einsum('bsd,md->bsm', q.astype(bf), mem_keys.astype(bf),
                   preferred_element_type=f32), axis=-1)
    mem_out = jnp.einsum('bsm,bmd->bsd', w_read.astype(bf), state.astype(bf),
                         preferred_element_type=f32)

    h = _layer_norm(mem_out, ln1_g, ln1_b)

    qh = (h @ attn_Wq).reshape(b, S, H, HD)
    kh = (h @ attn_Wk).reshape(b, S, H, HD)
    vh = (h @ attn_Wv).reshape(b, S, H, HD)
    bf = jnp.bfloat16
    scores = jnp.einsum('bqhd,bkhd->bhqk', qh.astype(bf), kh.astype(bf),
                        preferred_element_type=jnp.float32) / np.sqrt(HD)
    idx = jnp.arange(S)
    band = jnp.abs(idx[:, None] - idx[None, :]) < WIN
    scores = jnp.where(band[None, None], scores, jnp.finfo(scores.dtype).min)
    attn = jax.nn.softmax(scores, axis=-1)
    ao = jnp.einsum('bhqk,bkhd->bqhd', attn.astype(bf), vh.astype(bf),
                    preferred_element_type=jnp.float32).reshape(b, S, H * HD)
    ao = ao @ attn_Wo

    ao = _layer_norm(ao, ln2_g, ln2_b)
    return ao @ out_W + out_b


_JIT_CACHE = {}


def _get_fn():
    if "fn" in _JIT_CACHE:
        return _JIT_CACHE["fn"]

    devs = jax.devices()
    use_shard = len(devs) >= N_DEV
    if use_shard:
        try:
            from jax.sharding import Mesh, NamedSharding, PartitionSpec as Ps
            mesh = Mesh(np.array(devs[:N_DEV]).reshape(2, 4), ("b", "s"))
            repl = NamedSharding(mesh, Ps())
            x_sh = NamedSharding(mesh, Ps("b", "s", None))
            in_shardings = (x_sh,) + (repl,) * 15
            out_sharding = NamedSharding(mesh, Ps("b", "s", None))
            fn = jax.jit(_forward, in_shardings=in_shardings,
                         out_shardings=out_sharding)
            _JIT_CACHE["fn"] = fn
            _JIT_CACHE["sharded"] = True
            _JIT_CACHE["in_shardings"] = in_shardings
            return fn
        except Exception:
            pass
    fn = jax.jit(_forward)
    _JIT_CACHE["fn"] = fn
    _JIT_CACHE["sharded"] = False
    return fn


def kernel(**inputs) -> np.ndarray:
    fn = _get_fn()
    order = ["x", "persistent_memory", "mem_keys", "mem_Wk", "mem_Wv",
             "mem_Wq", "attn_Wq", "attn_Wk", "attn_Wv", "attn_Wo",
             "ln1_g", "ln1_b", "ln2_g", "ln2_b", "out_W", "out_b"]
    args = [jnp.asarray(inputs[name]) for name in order]
    out = None
    for attempt in range(2):  # axon can transiently fail ("mesh desynced")
        try:
            out = np.asarray(jax.block_until_ready(fn(*args)))
            break
        except Exception:
            continue
    if out is None:
        # Fallback: single-device jit (always correct).
        fn = jax.jit(_forward)
        _JIT_CACHE["fn"] = fn
        out = np.asarray(jax.block_until_ready(fn(*args)))
    return out.astype(np.float32)
